# revision 6
# baseline (speedup 1.0000x reference)
"""Trainium2 Bass kernel for nn_GraphNeuralNetwork_27728308863842.

The reference model's output is (policy_logits[10000], value[1]) and both
depend ONLY on the global-state path:
    g      = x_global @ glob_w + glob_b                      # [256]
    policy = relu(g @ pol_w1 + pol_b1) @ pol_w2 + pol_b2     # [10000]
    value  = tanh(relu(g @ val_w1 + val_b1) @ val_w2 + val_b2)
The node/edge message-passing loop never feeds the heads (the reference
notes it reproduces that faithfully), so it is dead code and is not
computed here.

Distribution: pol_w2 [256,10000] is column-sharded 8 ways (1250 cols per
core); everything else is replicated. Each core computes the full g/ph
vectors and its policy-logit slice; core 0's value is used.
"""

import sys

for _p in ("/opt/trn_rl_repo",):
    if _p not in sys.path:
        sys.path.append(_p)

import numpy as np

import concourse.bass as bass
import concourse.bacc as bacc
import concourse.mybir as mybir
from concourse.tile import TileContext
from concourse.bass_utils import run_bass_kernel_spmd

N_CORES = 8
IN_GLOB = 100
HID = 256
POLICY_DIM = 10000
VAL_HID = 128
SHARD = POLICY_DIM // N_CORES  # 1250

F32 = mybir.dt.float32

# Filled with the BassKernelResults of the most recent run (for test.py).
LAST_RESULTS = None


def _build_nc():
    # Bacc (not raw Bass): its compile() runs move_matmul_waits_to_ldweights
    # + generate_event_semaphores, required because TRN2 instructions carry
    # at most one semaphore wait.
    nc = bacc.Bacc(
        "TRN2", target_bir_lowering=False, debug=False, num_devices=N_CORES
    )

    # Per-core DRAM I/O. Host pre-reshapes so every DMA is a contiguous
    # 2D block landing in its final SBUF layout ([partition, free]).
    xg_d = nc.dram_tensor("xg", [IN_GLOB, 1], F32, kind="ExternalInput")
    gw_d = nc.dram_tensor("gw", [IN_GLOB, HID], F32, kind="ExternalInput")
    gb_d = nc.dram_tensor("gb", [128, 2], F32, kind="ExternalInput")
    pw1_d = nc.dram_tensor("pw1", [2, 128, HID], F32, kind="ExternalInput")
    pb1_d = nc.dram_tensor("pb1", [128, 2], F32, kind="ExternalInput")
    pw2_d = nc.dram_tensor("pw2", [2, 128, SHARD], F32, kind="ExternalInput")
    pb2_d = nc.dram_tensor("pb2", [1, SHARD], F32, kind="ExternalInput")
    vw1_d = nc.dram_tensor("vw1", [2, 128, VAL_HID], F32, kind="ExternalInput")
    vb1_d = nc.dram_tensor("vb1", [128, 1], F32, kind="ExternalInput")
    vw2_d = nc.dram_tensor("vw2", [VAL_HID, 1], F32, kind="ExternalInput")
    vb2_d = nc.dram_tensor("vb2", [1, 1], F32, kind="ExternalInput")

    pol_d = nc.dram_tensor("pol", [1, SHARD], F32, kind="ExternalOutput")
    val_d = nc.dram_tensor("val", [1, 1], F32, kind="ExternalOutput")

    ACT = mybir.ActivationFunctionType

    with TileContext(nc) as tc:
        with (
            tc.tile_pool(name="sbuf", bufs=1) as sb,
            tc.tile_pool(name="psum", bufs=1, space=bass.MemorySpace.PSUM) as pp,
            tc.tile_pool(name="psum_pol", bufs=2, space=bass.MemorySpace.PSUM) as ppol_pool,
        ):
            # ---- loads -------------------------------------------------
            xg = sb.tile([IN_GLOB, 1], F32, tag="xg")
            nc.sync.dma_start(out=xg[:], in_=xg_d[:])
            gw = sb.tile([IN_GLOB, HID], F32, tag="gw")
            nc.sync.dma_start(out=gw[:], in_=gw_d[:])
            gb = sb.tile([128, 2], F32, tag="gb")
            nc.sync.dma_start(out=gb[:], in_=gb_d[:])
            pb1 = sb.tile([128, 2], F32, tag="pb1")
            nc.sync.dma_start(out=pb1[:], in_=pb1_d[:])
            pb2 = sb.tile([1, SHARD], F32, tag="pb2")
            nc.sync.dma_start(out=pb2[:], in_=pb2_d[:])
            vb1 = sb.tile([128, 1], F32, tag="vb1")
            nc.sync.dma_start(out=vb1[:], in_=vb1_d[:])
            vw2 = sb.tile([VAL_HID, 1], F32, tag="vw2")
            nc.sync.dma_start(out=vw2[:], in_=vw2_d[:])
            vb2 = sb.tile([1, 1], F32, tag="vb2")
            nc.sync.dma_start(out=vb2[:], in_=vb2_d[:])

            pw1 = []
            pw2 = []
            vw1 = []
            for k in range(2):
                t = sb.tile([128, HID], F32, tag=f"pw1_{k}")
                nc.sync.dma_start(out=t[:], in_=pw1_d[k])
                pw1.append(t)
                t = sb.tile([128, SHARD], F32, tag=f"pw2_{k}")
                nc.sync.dma_start(out=t[:], in_=pw2_d[k])
                pw2.append(t)
                t = sb.tile([128, VAL_HID], F32, tag=f"vw1_{k}")
                nc.sync.dma_start(out=t[:], in_=vw1_d[k])
                vw1.append(t)

            # ---- g = x_global @ glob_w + glob_b, stored transposed -----
            # gT[:, j] = g[128j:128j+128]
            gT = sb.tile([128, 2], F32, tag="gT")
            for j in range(2):
                pg = pp.tile([128, 1], F32, tag=f"pg_{j}")
                nc.tensor.matmul(
                    pg[:], gw[:, j * 128:(j + 1) * 128], xg[:],
                    start=True, stop=True,
                )
                nc.vector.tensor_add(gT[:, j:j + 1], pg[:], gb[:, j:j + 1])

            # ---- ph = relu(g @ pol_w1 + pol_b1), stored transposed -----
            phT = sb.tile([128, 2], F32, tag="phT")
            for j in range(2):
                pph = pp.tile([128, 1], F32, tag=f"pph_{j}")
                for k in range(2):
                    nc.tensor.matmul(
                        pph[:], pw1[k][:, j * 128:(j + 1) * 128], gT[:, k:k + 1],
                        start=(k == 0), stop=(k == 1),
                    )
                nc.scalar.activation(
                    phT[:, j:j + 1], pph[:], ACT.Relu, bias=pb1[:, j:j + 1],
                )

            # ---- policy logits = ph @ pol_w2_shard + pol_b2_shard ------
            pol = sb.tile([1, SHARD], F32, tag="pol")
            n0 = 0
            while n0 < SHARD:
                nn = min(512, SHARD - n0)
                ppol = ppol_pool.tile([1, 512], F32, tag="ppol")
                for k in range(2):
                    nc.tensor.matmul(
                        ppol[:, :nn], phT[:, k:k + 1], pw2[k][:, n0:n0 + nn],
                        start=(k == 0), stop=(k == 1),
                    )
                nc.vector.tensor_add(
                    pol[:, n0:n0 + nn], ppol[:, :nn], pb2[:, n0:n0 + nn],
                )
                n0 += nn
            nc.sync.dma_start(out=pol_d[:], in_=pol[:])

            # ---- value head -------------------------------------------
            pvh = pp.tile([128, 1], F32, tag="pvh")
            for k in range(2):
                nc.tensor.matmul(
                    pvh[:], vw1[k][:], gT[:, k:k + 1],
                    start=(k == 0), stop=(k == 1),
                )
            vhT = sb.tile([VAL_HID, 1], F32, tag="vhT")
            nc.scalar.activation(vhT[:], pvh[:VAL_HID, :], ACT.Relu, bias=vb1[:])

            pval = pp.tile([1, 1], F32, tag="pval")
            nc.tensor.matmul(pval[:], vhT[:], vw2[:], start=True, stop=True)
            val = sb.tile([1, 1], F32, tag="val")
            nc.scalar.activation(val[:], pval[:], ACT.Tanh, bias=vb2[:])
            nc.sync.dma_start(out=val_d[:], in_=val[:])

    nc.compile()
    return nc


def _in_map_for_core(inputs, core):
    f32 = lambda a: np.ascontiguousarray(np.asarray(a, dtype=np.float32))
    pw2_shard = np.asarray(inputs["pol_w2"], np.float32)[
        :, core * SHARD:(core + 1) * SHARD
    ]
    pb2_shard = np.asarray(inputs["pol_b2"], np.float32)[
        core * SHARD:(core + 1) * SHARD
    ]
    return {
        "xg": f32(inputs["x_global"]).reshape(IN_GLOB, 1),
        "gw": f32(inputs["glob_w"]),
        "gb": f32(np.asarray(inputs["glob_b"], np.float32).reshape(2, 128).T),
        "pw1": f32(inputs["pol_w1"]).reshape(2, 128, HID),
        "pb1": f32(np.asarray(inputs["pol_b1"], np.float32).reshape(2, 128).T),
        "pw2": f32(pw2_shard).reshape(2, 128, SHARD),
        "pb2": f32(pb2_shard).reshape(1, SHARD),
        "vw1": f32(inputs["val_w1"]).reshape(2, 128, VAL_HID),
        "vb1": f32(inputs["val_b1"]).reshape(128, 1),
        "vw2": f32(inputs["val_w2"]).reshape(VAL_HID, 1),
        "vb2": f32(inputs["val_b2"]).reshape(1, 1),
    }


def kernel(**inputs):
    global LAST_RESULTS
    nc = _build_nc()
    in_maps = [_in_map_for_core(inputs, c) for c in range(N_CORES)]
    res = run_bass_kernel_spmd(nc, in_maps, list(range(N_CORES)))
    LAST_RESULTS = res
    pol = np.concatenate(
        [np.asarray(res.results[c]["pol"]).reshape(SHARD) for c in range(N_CORES)]
    ).astype(np.float32)
    val = np.asarray(res.results[0]["val"]).reshape(1).astype(np.float32)
    return pol, val


# revision 12
# speedup vs baseline: 1.2791x; 1.2791x over previous
"""Trainium2 Bass kernel for nn_GraphNeuralNetwork_27728308863842.

The reference model's output is (policy_logits[10000], value[1]) and both
depend ONLY on the global-state path:
    g      = x_global @ glob_w + glob_b                      # [256]
    policy = relu(g @ pol_w1 + pol_b1) @ pol_w2 + pol_b2     # [10000]
    value  = tanh(relu(g @ val_w1 + val_b1) @ val_w2 + val_b2)
The node/edge message-passing loop never feeds the heads (the reference
notes it reproduces that faithfully), so it is dead code and is not
computed here.

Distribution: pol_w2 [256,10000] is column-sharded 8 ways (1250 cols per
core); everything else is replicated. Each core computes the full g/ph
vectors and its policy-logit slice; core 0's value is used.

Layout/perf notes:
- All small weights ride in ONE host-packed [128,1030] f32 buffer (sm):
  one DMA issue instead of eleven (HWDGE issue is ~0.7us each, serialized
  per ring). glob_b is folded into the g matmul as an extra K row.
- pol_b2 is DMA'd straight into the policy PSUM banks; the policy
  matmuls accumulate on top (start=False), so no separate bias add.
- The value is written (tanh) into the same PSUM tile at column 1250, so
  one [1,1251] DMA stores both outputs.
- sm+pb2 issue on the Scalar(ACT) HWDGE ring, the two 640KB pol_w2
  chunks on the Sync ring: issue runs in parallel on both rings.
- A dummy relu early on ACT pulls the 1.3us ACT_TABLE_LOAD off the
  critical path.
"""

import sys

for _p in ("/opt/trn_rl_repo",):
    if _p not in sys.path:
        sys.path.append(_p)

import numpy as np

import concourse.bass as bass
import concourse.bacc as bacc
import concourse.mybir as mybir
from concourse.tile import TileContext
from concourse.bass_utils import run_bass_kernel_spmd

N_CORES = 8
IN_GLOB = 100
HID = 256
POLICY_DIM = 10000
VAL_HID = 128
SHARD = POLICY_DIM // N_CORES  # 1250

F32 = mybir.dt.float32

# Column offsets inside the packed small-weights buffer [128, SM_COLS].
_XG = 0            # [101, 1]   x_global ++ [1.0]
_GW = 1            # [101, 256] glob_w ++ glob_b row
_PW1 = 257         # [128, 512] pol_w1 as two 128-row chunks side by side
_PB1 = 769         # [128, 2]   pol_b1 chunks
_VW1 = 771         # [128, 256] val_w1 as two 128-row chunks
_VB1 = 1027        # [128, 1]
_VW2 = 1028        # [128, 1]
_VB2 = 1029        # [1, 1] at row 0
SM_COLS = 1030

# Filled with the BassKernelResults of the most recent run (for test.py).
LAST_RESULTS = None


def _build_nc():
    nc = bacc.Bacc(
        "TRN2", target_bir_lowering=False, debug=False, num_devices=N_CORES
    )

    sm_d = nc.dram_tensor("sm", [128, SM_COLS], F32, kind="ExternalInput")
    pw2a_d = nc.dram_tensor("pw2a", [128, SHARD], F32, kind="ExternalInput")
    pw2b_d = nc.dram_tensor("pw2b", [128, SHARD], F32, kind="ExternalInput")
    pb2_d = nc.dram_tensor("pb2", [1, SHARD], F32, kind="ExternalInput")
    out_d = nc.dram_tensor("out", [1, SHARD + 1], F32, kind="ExternalOutput")

    ACT = mybir.ActivationFunctionType

    with TileContext(nc) as tc:
        with (
            tc.tile_pool(name="sbuf", bufs=1) as sb,
            tc.tile_pool(name="psum", bufs=4, space=bass.MemorySpace.PSUM) as pp,
            tc.tile_pool(name="psum_pol", bufs=1, space=bass.MemorySpace.PSUM) as pq,
        ):
            # ---- loads: 2 on the ACT ring, 2 on the Sync ring ----------
            sm = sb.tile([128, SM_COLS], F32, tag="sm")
            nc.scalar.dma_start(out=sm[:], in_=sm_d[:])

            # Policy PSUM: 3 banks for the 1250 logits. PE accumulation
            # ignores externally-written PSUM contents, so pol_b2 is added
            # by DVE on the way out instead.
            ppol = pq.tile([1, 3 * 512], F32, tag="ppol")
            pb2 = sb.tile([1, SHARD], F32, tag="pb2")
            nc.scalar.dma_start(out=pb2[:], in_=pb2_d[:])

            pw2 = []
            for k, dram in ((0, pw2a_d), (1, pw2b_d)):
                t = sb.tile([128, SHARD], F32, tag=f"pw2_{k}")
                nc.sync.dma_start(out=t[:], in_=dram[:])
                pw2.append(t)

            # ---- ACT table prewarm (overlaps the DMA phase) ------------
            warm = sb.tile([128, 1], F32, tag="warm")
            nc.gpsimd.memset(warm[:], 0.0)
            warm2 = sb.tile([128, 1], F32, tag="warm2")
            nc.scalar.activation(warm2[:], warm[:], ACT.Relu)

            # ---- g = [x_global;1] @ [glob_w;glob_b], stored transposed -
            gT = sb.tile([128, 2], F32, tag="gT")
            for j in range(2):
                pg = pp.tile([128, 1], F32, tag="acc")
                nc.tensor.matmul(
                    pg[:], sm[0:101, _GW + j * 128:_GW + (j + 1) * 128],
                    sm[0:101, _XG:_XG + 1], start=True, stop=True,
                )
                nc.scalar.activation(gT[:, j:j + 1], pg[:], ACT.Copy)

            # ---- ph = relu(g @ pol_w1 + pol_b1), stored transposed -----
            phT = sb.tile([128, 2], F32, tag="phT")
            for j in range(2):
                pph = pp.tile([128, 1], F32, tag="acc")
                for k in range(2):
                    c = _PW1 + k * 256 + j * 128
                    nc.tensor.matmul(
                        pph[:], sm[:, c:c + 128], gT[:, k:k + 1],
                        start=(k == 0), stop=(k == 1),
                    )
                nc.scalar.activation(
                    phT[:, j:j + 1], pph[:], ACT.Relu,
                    bias=sm[:, _PB1 + j:_PB1 + j + 1],
                )

            # ---- policy: matmul chunks, bias added on the way out ------
            # DMA can't touch PSUM, so each bank is moved to pol_sb by an
            # otherwise-idle DVE (fused +pol_b2) as soon as it finishes.
            pol_sb = sb.tile([1, SHARD + 1], F32, tag="pol_sb")
            n0 = 0
            while n0 < SHARD:
                nn = min(512, SHARD - n0)
                for k in range(2):
                    nc.tensor.matmul(
                        ppol[:, n0:n0 + nn], phT[:, k:k + 1],
                        pw2[k][:, n0:n0 + nn],
                        start=(k == 0), stop=(k == 1),
                    )
                nc.vector.tensor_add(
                    pol_sb[:, n0:n0 + nn], ppol[:, n0:n0 + nn], pb2[:, n0:n0 + nn]
                )
                n0 += nn

            # ---- value head -> ppol[0, SHARD] --------------------------
            pvh = pp.tile([128, 1], F32, tag="acc")
            for k in range(2):
                c = _VW1 + k * 128
                nc.tensor.matmul(
                    pvh[:], sm[:, c:c + 128], gT[:, k:k + 1],
                    start=(k == 0), stop=(k == 1),
                )
            vhT = sb.tile([VAL_HID, 1], F32, tag="vhT")
            nc.scalar.activation(
                vhT[:], pvh[:VAL_HID, :], ACT.Relu,
                bias=sm[:VAL_HID, _VB1:_VB1 + 1],
            )
            pval = pp.tile([1, 1], F32, tag="acc")
            nc.tensor.matmul(
                pval[:], vhT[:], sm[:VAL_HID, _VW2:_VW2 + 1],
                start=True, stop=True,
            )
            nc.scalar.activation(
                pol_sb[:, SHARD:SHARD + 1], pval[:], ACT.Tanh,
                bias=sm[0:1, _VB2:_VB2 + 1],
            )

            # ---- single store: 1250 logits + value ---------------------
            nc.sync.dma_start(out=out_d[:], in_=pol_sb[:])

    nc.compile()
    return nc


def _in_map_for_core(inputs, core):
    f32 = lambda a: np.asarray(a, dtype=np.float32)
    sm = np.zeros((128, SM_COLS), np.float32)
    sm[0:100, _XG] = f32(inputs["x_global"])
    sm[100, _XG] = 1.0
    sm[0:100, _GW:_GW + HID] = f32(inputs["glob_w"])
    sm[100, _GW:_GW + HID] = f32(inputs["glob_b"])
    pw1 = f32(inputs["pol_w1"])
    sm[:, _PW1:_PW1 + 256] = pw1[0:128]
    sm[:, _PW1 + 256:_PW1 + 512] = pw1[128:256]
    sm[:, _PB1:_PB1 + 2] = f32(inputs["pol_b1"]).reshape(2, 128).T
    vw1 = f32(inputs["val_w1"])
    sm[:, _VW1:_VW1 + 128] = vw1[0:128]
    sm[:, _VW1 + 128:_VW1 + 256] = vw1[128:256]
    sm[0:VAL_HID, _VB1] = f32(inputs["val_b1"])
    sm[0:VAL_HID, _VW2] = f32(inputs["val_w2"]).reshape(VAL_HID)
    sm[0, _VB2] = f32(inputs["val_b2"]).reshape(())

    pw2 = f32(inputs["pol_w2"])[:, core * SHARD:(core + 1) * SHARD]
    return {
        "sm": sm,
        "pw2a": np.ascontiguousarray(pw2[0:128]),
        "pw2b": np.ascontiguousarray(pw2[128:256]),
        "pb2": np.ascontiguousarray(
            f32(inputs["pol_b2"])[core * SHARD:(core + 1) * SHARD].reshape(1, SHARD)
        ),
    }


def kernel(**inputs):
    global LAST_RESULTS
    nc = _build_nc()
    in_maps = [_in_map_for_core(inputs, c) for c in range(N_CORES)]
    res = run_bass_kernel_spmd(nc, in_maps, list(range(N_CORES)))
    LAST_RESULTS = res
    pol = np.concatenate(
        [np.asarray(res.results[c]["out"]).reshape(SHARD + 1)[:SHARD]
         for c in range(N_CORES)]
    ).astype(np.float32)
    val = np.asarray(res.results[0]["out"]).reshape(SHARD + 1)[SHARD:].astype(
        np.float32
    )
    return pol, val
